# revision 16
# baseline (speedup 1.0000x reference)
"""GIN message-passing network on 8 Trainium2 NeuronCores (Bass/Tile).

Strategy:
  - Nodes are split into 8 contiguous ranges at graph boundaries (so mean/sum
    pooling is core-local). Edges are owned by the core owning their dst node.
  - Each core keeps a full copy of node features h in HBM for gathering,
    split into NCHUNK row-chunk tables (chunk q = blocks cb[q]..cb[q+1] of
    every owner). After each conv, NCHUNK chunked AllGathers rebuild the
    tables; chunk q's collective fires as soon as chunk q's new features are
    written, and the next conv's phase-q gathers wait only on chunk q —
    overlapping the collective with the previous/next conv's gather drain.
  - Aggregation (segment-sum over incoming edges): gpsimd dma_gather pulls
    h[src] rows in fixed 1024-index calls rotating over the 4 SWDGE queues
    (single_packet=True: descriptors bundle into packets, ~15% faster drain).
    Edges are sorted by (src chunk phase, dst block); indices are int16
    relative to a per-chunk mid-table base. Preprocessing guarantees every
    call ends on a non-negative index (the gather ucode faults on trailing
    negatives). A DVE is_equal against an iota row builds each chunk's
    [128 edge x 128 node] one-hot, and PE matmuls Mg.T @ onehot accumulate
    agg^T in PSUM per 128-node dst block; later phases DVE-add into zT.
  - The GIN MLP runs transposed (features on partitions) in bf16 so
    biases+ReLU fuse as per-partition scalar.activation; a PE transpose
    yields node-major h_new for the next round's gather table.
  - Pooling: one-hot graph matmul accumulated over all blocks, then the
    post-MLP, all on-device; host reassembles the [256, 128] output.
"""

import os
import numpy as np
import ml_dtypes

N = 50000
E = 800000
NF = 9
EMB = 128
HID = 256
L = 3
NUM_CONVS = 2
G = 256
NCORES = 8
P = 128


def _preprocess(x, edge_index, batch, nchunk, mlpg, gch):
    """Host-side graph partitioning and phase-chunked edge layout."""
    gstart = np.searchsorted(batch, np.arange(G + 1))  # [G+1]

    # core graph splits balancing node counts
    gs = [0]
    for c in range(1, NCORES):
        t = (c * N) // NCORES
        i = int(np.searchsorted(gstart, t))
        if i > 0 and (i >= G + 1 or abs(int(gstart[i - 1]) - t) <= abs(int(gstart[i]) - t)):
            i -= 1
        i = max(gs[-1] + 1, min(i, G - (NCORES - c)))
        gs.append(i)
    gs.append(G)
    gs = np.array(gs, np.int64)
    ns = gstart[gs]  # node split points, ns[0]=0, ns[8]=N

    ncounts = np.diff(ns)
    NPAD = int(-(-ncounts.max() // P) * P)
    NB = NPAD // P

    # chunk boundaries in blocks, group-aligned. GNN_CB overrides cb[1] for
    # the asymmetric big-chunk/early-collective split.
    cb = [0]
    cbenv = os.environ.get("GNN_CB")
    if nchunk == 2 and cbenv:
        cb.append(min(((int(cbenv) // mlpg) * mlpg) or mlpg, NB - 1))
    else:
        for q in range(1, nchunk):
            v = ((q * NB) // nchunk // mlpg) * mlpg
            v = max(v, cb[-1] + mlpg)
            cb.append(min(v, NB))
    cb.append(NB)
    cb = np.array(cb, np.int64)
    CR = np.diff(cb) * P            # rows per chunk (per owner)
    ROWS_q = NCORES * CR            # rows per chunk table
    BASE_q = ROWS_q // 2

    chunk_of_block = np.searchsorted(cb, np.arange(NB), side="right") - 1

    node_ids = np.arange(N, dtype=np.int64)
    node_owner = np.searchsorted(ns, node_ids, side="right") - 1
    loc = node_ids - ns[node_owner]
    blkl = loc >> 7
    node_q = chunk_of_block[blkl]
    node_row = node_owner * CR[node_q] + (loc - cb[node_q] * P)

    src = np.asarray(edge_index[0], np.int64)
    dst = np.asarray(edge_index[1], np.int64)
    src_q = node_q[src]
    src_row = node_row[src]
    dst_owner = node_owner[dst]
    dl_all = dst - ns[dst_owner]
    blk_all = dl_all >> 7

    # per-(core, phase, block) counts -> shared chunk counts K[q][b]
    cnt = np.zeros((NCORES, nchunk, NB), np.int64)
    np.add.at(cnt, (dst_owner, src_q, blk_all), 1)
    K_qb = (-(-cnt.max(axis=0) // P)).astype(np.int64)  # [nchunk, NB]
    # every block needs at least one chunk overall (pads handle empties)
    empt = K_qb.sum(axis=0) == 0
    K_qb[0][empt] = 1
    # global chunk offsets: phases concatenated
    o_flat = np.concatenate([[0], np.cumsum(K_qb.reshape(-1))])
    o_qb = o_flat[:-1].reshape(nchunk, NB)       # [nchunk, NB]
    CHT_q = K_qb.sum(axis=1)                      # chunks per phase
    pstart = np.concatenate([[0], np.cumsum(CHT_q)])  # phase chunk offsets
    CHT = int(pstart[-1])
    NIDX = CHT * P

    # fixed-size gather calls per phase: (global chunk start, n chunks)
    calls = []   # list per phase
    for q in range(nchunk):
        cl = []
        c0 = int(pstart[q])
        while c0 < pstart[q + 1]:
            cn = int(min(gch, pstart[q + 1] - c0))
            cl.append((c0, cn))
            c0 += cn
        calls.append(cl)

    blk_of_chunk = np.empty(CHT, np.int64)
    for q in range(nchunk):
        blk_of_chunk[pstart[q]:pstart[q + 1]] = np.repeat(np.arange(NB), K_qb[q])

    per_core = []
    for c in range(NCORES):
        m = dst_owner == c
        sq = src_q[m]
        srow = src_row[m]
        rel = srow - BASE_q[sq]
        dl = dl_all[m]
        blk = blk_all[m]
        din = dl & 127
        order = np.lexsort(((rel >= 0), blk, sq))  # phase, block, neg-rel first
        rel, din, blk, sq = rel[order], din[order], blk[order], sq[order]

        # rank within (phase, block) segment
        seg = sq * NB + blk
        ccnt = np.bincount(seg, minlength=nchunk * NB)
        first = np.concatenate([[0], np.cumsum(ccnt)])[:-1]
        rank = np.arange(len(rel)) - first[seg]
        pos = o_qb[sq, blk] * P + rank

        flat_rel = np.zeros(NIDX, np.int32)  # pads: rel=0 (row BASE, valid)
        flat_rel[pos] = rel
        flat_din = np.full(NIDX, -1.0, np.float32)
        flat_din[pos] = din.astype(np.float32)
        # trailing-trim guard: each gather CALL must end on a non-negative
        # index (the HW chokes on trailing negatives). Swap the call-end slot
        # with a non-negative slot of the same (phase, block) segment.
        call_ends = set()
        for q in range(nchunk):
            for (c0, cn) in calls[q]:
                call_ends.add((c0 + cn) * P - 1)
        for end in sorted(call_ends):
            if flat_rel[end] >= 0:
                continue
            ch = end // P
            b = int(blk_of_chunk[ch])
            q = int(np.searchsorted(pstart, ch, side="right") - 1)
            seg0 = int(o_qb[q, b]) * P
            seg1 = seg0 + int(K_qb[q, b]) * P
            cand = np.nonzero(flat_rel[seg0:seg1] >= 0)[0]
            cand = [seg0 + int(j) for j in cand
                    if (seg0 + int(j)) not in call_ends]
            assert cand, "no non-negative slot available in segment"
            j = cand[-1]
            flat_rel[[j, end]] = flat_rel[[end, j]]
            flat_din[[j, end]] = flat_din[[end, j]]

        assert flat_rel.min() >= -32768 and flat_rel.max() < 32768
        idx16 = flat_rel.astype(np.int16).reshape(-1, 16).T  # [16, NIDX/16]
        idx_np = np.tile(idx16, (8, 1)).copy()  # [128, NIDX/16]
        dstloc_np = flat_din.reshape(CHT, P).T.copy()  # [128, CHT]

        # pooling one-hot + inverse counts
        ng = int(gs[c + 1] - gs[c])
        assert ng <= P
        bl = batch[ns[c]:ns[c + 1]] - gs[c]
        n_c = int(ncounts[c])
        ohg = np.zeros((NPAD, P), np.float32)
        ohg[np.arange(n_c), bl] = 1.0
        ohg_t = ohg.reshape(NB, P, P).transpose(1, 0, 2).reshape(P, NB * P).astype(ml_dtypes.bfloat16)
        cnts = np.bincount(bl, minlength=P)[:P]
        invc = np.zeros((P, 1), np.float32)
        invc[:ng, 0] = 1.0 / np.maximum(cnts[:ng], 1)

        per_core.append(dict(idx=idx_np, dstloc=dstloc_np, ohg=ohg_t, invc=invc,
                             ng=ng, n_c=n_c))

    # initial h chunk tables (bf16, same layout as the allgather outputs)
    h0q = []
    for q in range(nchunk):
        t = np.zeros((int(ROWS_q[q]), EMB), np.float32)
        for c in range(NCORES):
            r0 = int(cb[q]) * P
            r1 = min(int(cb[q + 1]) * P, int(ncounts[c]))
            if r1 > r0:
                t[c * int(CR[q]):c * int(CR[q]) + (r1 - r0), :NF] = \
                    x[ns[c] + r0:ns[c] + r1]
        h0q.append(t.astype(ml_dtypes.bfloat16))

    geom = dict(NPAD=NPAD, NB=NB, cb=cb, CR=CR, ROWS_q=ROWS_q, BASE_q=BASE_q,
                K_qb=K_qb, o_qb=o_qb, CHT=CHT, NIDX=NIDX, pstart=pstart,
                calls=calls, ns=ns, gs=gs, nchunk=nchunk)
    return geom, per_core, h0q


def _pack_weights(gin_w1, gin_b1, gin_w2, gin_b2, post_w1, post_b1, post_w2,
                  post_b2):
    w1 = np.concatenate([gin_w1[l] for l in range(L)], axis=1)  # [128, 768]
    w2 = np.concatenate(
        [gin_w2[l][h * P:(h + 1) * P, :] for l in range(L) for h in (0, 1)],
        axis=1)  # [128, 768]
    b1 = np.stack([gin_b1[l][h * P:(h + 1) * P] for l in range(L) for h in (0, 1)],
                  axis=1)  # [128, 6]
    b2 = np.stack([gin_b2[l] for l in range(L)], axis=1)  # [128, 3]
    pw1 = np.concatenate(
        [post_w1[kc * P:(kc + 1) * P, mh * P:(mh + 1) * P]
         for kc in (0, 1) for mh in (0, 1)], axis=1)  # [128, 512]
    pw2 = np.concatenate([post_w2[kc * P:(kc + 1) * P, :] for kc in (0, 1)],
                         axis=1)  # [128, 256]
    pb1 = np.stack([post_b1[mh * P:(mh + 1) * P] for mh in (0, 1)], axis=1)
    pb2 = post_b2[:, None]
    return dict(w1=w1, w2=w2, b1=b1, b2=b2, pw1=pw1, pw2=pw2, pb1=pb1, pb2=pb2)


def _build_program(geom, n_convs, reps=1):
    import concourse.bass as bass
    import concourse.bacc as bacc
    import concourse.tile as tile
    import concourse.mybir as mybir
    from concourse.masks import make_identity

    F32 = mybir.dt.float32
    BF16 = mybir.dt.bfloat16
    I16 = mybir.dt.int16
    Relu = mybir.ActivationFunctionType.Relu

    NPAD, NB = geom["NPAD"], geom["NB"]
    cb, CR, ROWS_q, BASE_q = geom["cb"], geom["CR"], geom["ROWS_q"], geom["BASE_q"]
    K_qb, o_qb, CHT, NIDX = geom["K_qb"], geom["o_qb"], geom["CHT"], geom["NIDX"]
    calls, pstart = geom["calls"], geom["pstart"]
    nchunk = geom["nchunk"]

    n_queues = int(os.environ.get("GNN_GQ", "4"))
    OHG = int(os.environ.get("GNN_OHG", "4"))  # onehot chunks per DVE op
    MLPG = 4     # 128-node blocks per MLP group (moving dim 512)
    GCH = int(os.environ.get("GNN_GCH", "8"))  # chunks per gather call
    single_packet = os.environ.get("GNN_SP", "1") == "1"
    cc_delay = int(os.environ.get("GNN_CCDELAY", "1"))
    # calls per conv pre-generated (prepare_only) during the previous conv's
    # collective stall; fired by trigger_dma once the table lands
    prep_n = int(os.environ.get("GNN_PREP", "0"))

    ndev = int(os.environ.get("GNN_NDEV", str(NCORES)))
    no_cc = os.environ.get("GNN_NO_CC", "0") == "1"
    nc = bacc.Bacc("TRN2", target_bir_lowering=False, debug=False,
                   enable_asserts=True, num_devices=ndev,
                   num_swdge_queues=4,
                   dynamic_dma_scratch_size=int(os.environ.get(
                       "GNN_DMA_SCRATCH", "49152")))

    t_h0q = [nc.dram_tensor(f"t_h0q{q}", [int(ROWS_q[q]), EMB], BF16,
                            kind="ExternalInput") for q in range(nchunk)]
    t_h0T = nc.dram_tensor("t_h0T", [P, NPAD], BF16, kind="ExternalInput")
    t_idx = nc.dram_tensor("t_idx", [P, NIDX // 16], I16, kind="ExternalInput")
    t_dstloc = nc.dram_tensor("t_dstloc", [P, CHT], F32, kind="ExternalInput")
    t_iota = nc.dram_tensor("t_iota", [P, OHG * P], F32, kind="ExternalInput")
    t_ohg = nc.dram_tensor("t_ohg", [P, NB * P], BF16, kind="ExternalInput")
    t_invc = nc.dram_tensor("t_invc", [P, 1], F32, kind="ExternalInput")
    t_w1 = nc.dram_tensor("t_w1", [P, L * 2 * P], BF16, kind="ExternalInput")
    t_w2 = nc.dram_tensor("t_w2", [P, L * 2 * P], BF16, kind="ExternalInput")
    t_b1 = nc.dram_tensor("t_b1", [P, L * 2], F32, kind="ExternalInput")
    t_b2 = nc.dram_tensor("t_b2", [P, L], F32, kind="ExternalInput")
    t_pw1 = nc.dram_tensor("t_pw1", [P, 4 * P], F32, kind="ExternalInput")
    t_pw2 = nc.dram_tensor("t_pw2", [P, 2 * P], F32, kind="ExternalInput")
    t_pb1 = nc.dram_tensor("t_pb1", [P, 2], F32, kind="ExternalInput")
    t_pb2 = nc.dram_tensor("t_pb2", [P, 1], F32, kind="ExternalInput")
    o_outT = nc.dram_tensor("o_outT", [P, P], F32, kind="ExternalOutput")

    # MLP block groups
    groups = []
    b0 = 0
    while b0 < NB:
        groups.append((b0, min(b0 + MLPG, NB)))
        b0 += MLPG

    qsems = [nc.alloc_semaphore(f"gsem{q}") for q in range(n_queues)]

    with tile.TileContext(nc) as tc:
        with tc.tile_pool(name="const", bufs=1) as cp, \
             tc.tile_pool(name="mgp", bufs=int(os.environ.get("GNN_MGB", str(8 + prep_n)))) as mgp, \
             tc.tile_pool(name="work", bufs=2) as wp, \
             tc.tile_pool(name="oh", bufs=4) as ohp, \
             tc.tile_pool(name="psA", bufs=2, space="PSUM") as psA, \
             tc.tile_pool(name="psT", bufs=1, space="PSUM") as psT_pool, \
             tc.tile_pool(name="psB", bufs=2, space="PSUM") as psB, \
             tc.tile_pool(name="psM", bufs=1, space="PSUM") as psM, \
             tc.tile_pool(name="psC", bufs=1, space="PSUM") as psC, \
             tc.tile_pool(name="dram", bufs=1, space="DRAM") as dram:

            idx_sb = cp.tile([P, NIDX // 16], I16)
            dstloc_sb = cp.tile([P, CHT], F32)
            iota_sb = cp.tile([P, OHG * P], F32)
            ohg_sb = cp.tile([P, NB * P], BF16)
            invc_sb = cp.tile([P, 1], F32)
            w1_sb = cp.tile([P, L * 2 * P], BF16)
            w2_sb = cp.tile([P, L * 2 * P], BF16)
            b1_sb = cp.tile([P, L * 2], F32)
            b2_sb = cp.tile([P, L], F32)
            pw1_sb = cp.tile([P, 4 * P], F32)
            pw2_sb = cp.tile([P, 2 * P], F32)
            pb1_sb = cp.tile([P, 2], F32)
            pb2_sb = cp.tile([P, 1], F32)
            ident = cp.tile([P, P], F32)
            ident_bf = cp.tile([P, P], BF16)
            for sb_t, dr_t in [(idx_sb, t_idx), (dstloc_sb, t_dstloc),
                               (iota_sb, t_iota), (ohg_sb, t_ohg),
                               (invc_sb, t_invc), (w1_sb, t_w1), (w2_sb, t_w2),
                               (b1_sb, t_b1), (b2_sb, t_b2), (pw1_sb, t_pw1),
                               (pw2_sb, t_pw2), (pb1_sb, t_pb1),
                               (pb2_sb, t_pb2)]:
                nc.sync.dma_start(sb_t[:], dr_t[:])
            make_identity(nc, ident[:])
            make_identity(nc, ident_bf[:])

            # persistent feature-major h (ping-pong) + zT staging
            hT0 = cp.tile([P, NPAD], BF16)
            hT1 = cp.tile([P, NPAD], BF16)
            hT_pp = [hT0, hT1]
            zT_all = cp.tile([P, NPAD], BF16)
            nc.sync.dma_start(hT0[:], t_h0T[:])

            hnew0 = dram.tile([NPAD, EMB], BF16)
            hnew1 = dram.tile([NPAD, EMB], BF16)
            hnew_pp = [hnew0, hnew1]
            n_cc = max(reps * n_convs - 1, 1)
            hfq_cv = [[dram.tile([int(ROWS_q[q]), EMB], BF16,
                                 addr_space="Shared", name=f"hf{i}q{q}")
                       for q in range(nchunk)] for i in range(n_cc)]

            psum_pool = psC.tile([P, P], F32, space="PSUM", tag="pool")

            def src_tab_for(gc):
                return [t_h0q[q] if (gc == 0 or no_cc) else
                        hfq_cv[gc - 1][q] for q in range(nchunk)]

            # per-conv gather-call state (survives across the conv loop so a
            # boundary can pre-generate the next conv's descriptors)
            states = {}

            def get_state(gc):
                if gc not in states:
                    states[gc] = dict(next_call=[0] * nchunk, chunk2mg={},
                                      ncalls=0, trig=set())
                return states[gc]

            def issue_one_call(gc, q, prep):
                st = get_state(gc)
                c0, cn = calls[q][st["next_call"][q]]
                mg = mgp.tile([P, GCH, P], BF16, tag="mg",
                              name=f"mg_{gc}_{q}_{c0}")
                qn = st["ncalls"] % n_queues
                kw = dict(prepare_only=True, sem=qsems[qn]) if prep else {}
                nc.gpsimd.dma_gather(
                    out_ap=mg[:, :cn, :],
                    in_ap=src_tab_for(gc)[q][int(BASE_q[q]):, :],
                    idxs_ap=idx_sb[:, c0 * 8:(c0 + cn) * 8],
                    num_idxs=cn * P,
                    num_idxs_reg=cn * P,
                    elem_size=EMB,
                    single_packet=single_packet,
                    queue_num=qn,
                    **kw,
                )
                if prep:
                    st["trig"].add(qn)
                st["ncalls"] += 1
                for j in range(cn):
                    st["chunk2mg"][c0 + j] = (mg, j)
                st["next_call"][q] += 1

            for gc in range(reps * n_convs):
                r, c = divmod(gc, n_convs)
                l = min(c // NUM_CONVS, L - 1)
                hT_cur = hT_pp[gc % 2]
                hT_nxt = hT_pp[(gc + 1) % 2]
                last = gc == reps * n_convs - 1

                st_gc = get_state(gc)
                # fire any descriptors pre-generated at the previous boundary
                for qn in sorted(st_gc["trig"]):
                    nc.gpsimd.trigger_dma(count=None, queue_num=qn)
                st_gc["trig"].clear()
                chunk2mg = st_gc["chunk2mg"]

                def issue_calls(q, need_end):
                    st = st_gc
                    while (st["next_call"][q] < len(calls[q])
                           and calls[q][st["next_call"][q]][0] < need_end):
                        issue_one_call(gc, q, False)

                def agg_block(q, b, first):
                    # psum-accumulate phase q of block b, then fold into zT
                    kb = int(K_qb[q][b])
                    bs = slice(b * P, (b + 1) * P)
                    if kb == 0:
                        if first:
                            nc.vector.tensor_copy(out=zT_all[:, bs],
                                                  in_=hT_cur[:, bs])
                        return
                    ob = int(o_qb[q][b])
                    issue_calls(q, ob + kb)
                    psumA = psA.tile([P, P], F32, space="PSUM", tag="agg",
                                     name=f"agg_{gc}_{q}_{b}")
                    n_oh = (kb + OHG - 1) // OHG
                    ohts = []
                    for j in range(n_oh):
                        k0 = j * OHG
                        kn = min(OHG, kb - k0)
                        oht = ohp.tile([P, OHG, P], BF16, tag="oh",
                                       name=f"oh_{gc}_{q}_{b}_{j}")
                        nc.vector.tensor_tensor(
                            out=oht[:, :kn, :],
                            in0=iota_sb[:, :kn * P].rearrange(
                                "p (a b) -> p a b", b=P),
                            in1=dstloc_sb[:, ob + k0:ob + k0 + kn]
                                .to_broadcast([P, kn, P]),
                            op=mybir.AluOpType.is_equal)
                        ohts.append((oht, k0, kn))
                    for oht, k0, kn in ohts:
                        for kk in range(kn):
                            k = k0 + kk
                            mg, off = chunk2mg[ob + k]
                            nc.tensor.matmul(out=psumA[:],
                                             lhsT=mg[:, off, :],
                                             rhs=oht[:, kk, :],
                                             start=(k == 0),
                                             stop=(k == kb - 1))
                    nc.vector.tensor_add(
                        out=zT_all[:, bs], in0=psumA[:],
                        in1=hT_cur[:, bs] if first else zT_all[:, bs])

                # small trailing phases first (their tables landed early via
                # the tiny tail collectives); phase 0 fused with the MLP
                for q in range(1, nchunk):
                    for b in range(NB):
                        agg_block(q, b, q == 1)

                pending_cc = []  # (chunk idx, groups countdown)
                for gi, (g0, g1) in enumerate(groups):
                    for b in range(g0, g1):
                        agg_block(0, b, nchunk == 1)
                    # grouped MLP: moving dim = 128 * (g1 - g0)
                    gw = (g1 - g0) * P
                    gsl = slice(g0 * P, g0 * P + gw)
                    z1 = []
                    for mh in range(2):
                        ps1 = psB.tile([P, 512], F32, space="PSUM", tag="mm1",
                                       name=f"mm1_{gc}_{g0}_{mh}")
                        nc.tensor.matmul(
                            out=ps1[:, :gw],
                            lhsT=w1_sb[:, (l * 2 + mh) * P:(l * 2 + mh + 1) * P],
                            rhs=zT_all[:, gsl], start=True, stop=True)
                        z1t = wp.tile([P, 512], BF16, tag=f"z1_{mh}",
                                      name=f"z1_{gc}_{g0}_{mh}")
                        nc.scalar.activation(
                            out=z1t[:, :gw], in_=ps1[:, :gw], func=Relu,
                            bias=b1_sb[:, l * 2 + mh:l * 2 + mh + 1])
                        z1.append(z1t)
                    ps2 = psM.tile([P, 512], F32, space="PSUM", tag="mm2",
                                   name=f"mm2_{gc}_{g0}")
                    for mh in range(2):
                        nc.tensor.matmul(
                            out=ps2[:, :gw],
                            lhsT=w2_sb[:, (l * 2 + mh) * P:(l * 2 + mh + 1) * P],
                            rhs=z1[mh][:, :gw], start=(mh == 0), stop=(mh == 1))
                    nc.scalar.activation(out=hT_nxt[:, gsl], in_=ps2[:, :gw],
                                         func=Relu, bias=b2_sb[:, l:l + 1])
                    # node-major h_new per block (for allgather / pooling)
                    for b in range(g0, g1):
                        bs = slice(b * P, (b + 1) * P)
                        psT = psT_pool.tile([P, P], BF16, space="PSUM", tag="tp",
                                            name=f"tp_{gc}_{b}")
                        nc.tensor.transpose(out=psT[:], in_=hT_nxt[:, bs],
                                            identity=ident_bf[:])
                        hnode = wp.tile([P, P], BF16, tag="hnode",
                                        name=f"hn_{gc}_{b}")
                        nc.scalar.copy(out=hnode[:], in_=psT[:])
                        if not last:
                            nc.sync.dma_start(hnew_pp[gc % 2][bs, :], hnode[:])
                        else:
                            nc.tensor.matmul(out=psum_pool[:],
                                             lhsT=ohg_sb[:, bs], rhs=hnode[:],
                                             start=(b == 0), stop=(b == NB - 1),
                                             skip_group_check=True)
                    # chunked allgathers, delayed a few groups so the hnew
                    # writes they wait on have drained (Pool queue-head stall)
                    if not last and not no_cc:
                        for q in range(nchunk):
                            if cb[q + 1] == g1:
                                pending_cc.append([q, cc_delay])
                        for pc in pending_cc:
                            pc[1] -= 1
                        while pending_cc and (pending_cc[0][1] < 0
                                              or gi == len(groups) - 1):
                            q = pending_cc.pop(0)[0]
                            nc.gpsimd.collective_compute(
                                "AllGather", mybir.AluOpType.bypass,
                                replica_groups=[list(range(NCORES))],
                                ins=[hnew_pp[gc % 2][int(cb[q]) * P:
                                                     int(cb[q + 1]) * P, :].opt()],
                                outs=[hfq_cv[gc][q].opt()])
                # pre-generate the next conv's first descriptors while the
                # collective runs (descriptor gen reads only idx metadata; the
                # deferred table read lands on trigger_dma)
                if not last:
                    st_nx = get_state(gc + 1)
                    for _ in range(prep_n):
                        if st_nx["next_call"][0] >= len(calls[0]):
                            break
                        issue_one_call(gc + 1, 0, True)

            # pooling epilogue
            sums_sb = cp.tile([P, P], F32)
            means_sb = cp.tile([P, P], F32)
            nc.vector.tensor_copy(out=sums_sb[:], in_=psum_pool[:])
            nc.vector.tensor_scalar(out=means_sb[:], in0=psum_pool[:],
                                    scalar1=invc_sb[:, 0:1], scalar2=None,
                                    op0=mybir.AluOpType.mult)
            psTs = psT_pool.tile([P, P], F32, space="PSUM", tag="tp")
            nc.tensor.transpose(out=psTs[:], in_=sums_sb[:], identity=ident[:])
            sT = cp.tile([P, P], F32)
            nc.scalar.copy(out=sT[:], in_=psTs[:])
            psTm = psT_pool.tile([P, P], F32, space="PSUM", tag="tp")
            nc.tensor.transpose(out=psTm[:], in_=means_sb[:], identity=ident[:])
            mT = cp.tile([P, P], F32)
            nc.scalar.copy(out=mT[:], in_=psTm[:])

            z1p = []
            for mh in range(2):
                ps3 = psB.tile([P, 512], F32, space="PSUM", tag="mm1")
                nc.tensor.matmul(out=ps3[:, :P],
                                 lhsT=pw1_sb[:, (0 * 2 + mh) * P:(0 * 2 + mh + 1) * P],
                                 rhs=sT[:], start=True, stop=False)
                nc.tensor.matmul(out=ps3[:, :P],
                                 lhsT=pw1_sb[:, (1 * 2 + mh) * P:(1 * 2 + mh + 1) * P],
                                 rhs=mT[:], start=False, stop=True)
                z1t = cp.tile([P, P], F32, name=f"z1p_{mh}")
                nc.scalar.activation(out=z1t[:], in_=ps3[:, :P], func=Relu,
                                     bias=pb1_sb[:, mh:mh + 1])
                z1p.append(z1t)
            ps4 = psM.tile([P, 512], F32, space="PSUM", tag="mm2")
            for kc in range(2):
                nc.tensor.matmul(out=ps4[:, :P], lhsT=pw2_sb[:, kc * P:(kc + 1) * P],
                                 rhs=z1p[kc][:], start=(kc == 0), stop=(kc == 1))
            out_sb = cp.tile([P, P], F32)
            nc.vector.tensor_scalar(out=out_sb[:], in0=ps4[:, :P],
                                    scalar1=pb2_sb[:, 0:1], scalar2=None,
                                    op0=mybir.AluOpType.add)
            nc.sync.dma_start(o_outT[:], out_sb[:])

    nc.compile()
    return nc


def kernel(**inputs):
    x = np.asarray(inputs["x"], np.float32)
    edge_index = np.asarray(inputs["edge_index"], np.int64)
    batch = np.asarray(inputs["batch"], np.int64)
    gin_w1 = np.asarray(inputs["gin_w1"], np.float32)
    gin_b1 = np.asarray(inputs["gin_b1"], np.float32)
    gin_w2 = np.asarray(inputs["gin_w2"], np.float32)
    gin_b2 = np.asarray(inputs["gin_b2"], np.float32)
    post_w1 = np.asarray(inputs["post_w1"], np.float32)
    post_b1 = np.asarray(inputs["post_b1"], np.float32)
    post_w2 = np.asarray(inputs["post_w2"], np.float32)
    post_b2 = np.asarray(inputs["post_b2"], np.float32)

    nchunk = int(os.environ.get("GNN_NCHUNK", "1"))
    MLPG = 4
    GCH = int(os.environ.get("GNN_GCH", "8"))
    geom, per_core, h0q = _preprocess(x, edge_index, batch, nchunk, MLPG, GCH)
    w = _pack_weights(gin_w1, gin_b1, gin_w2, gin_b2, post_w1, post_b1,
                      post_w2, post_b2)

    n_convs = int(os.environ.get("GNN_CONVS", L * NUM_CONVS))
    nc = _build_program(geom, n_convs, reps=int(os.environ.get('GNN_REPS', '1')))

    NPAD = geom["NPAD"]
    ns = geom["ns"]
    iota_np = np.tile(np.arange(128, dtype=np.float32),
                      (128, int(os.environ.get("GNN_OHG", "4"))))
    w1_bf = w["w1"].astype(ml_dtypes.bfloat16)
    w2_bf = w["w2"].astype(ml_dtypes.bfloat16)
    in_maps = []
    for c in range(NCORES):
        pc = per_core[c]
        h0T = np.zeros((P, NPAD), np.float32)
        n_c = int(ns[c + 1] - ns[c])
        h0T[:NF, :n_c] = x[ns[c]:ns[c + 1]].T
        im = {
            "t_h0T": h0T.astype(ml_dtypes.bfloat16), "t_idx": pc["idx"],
            "t_dstloc": pc["dstloc"], "t_iota": iota_np, "t_ohg": pc["ohg"],
            "t_invc": pc["invc"], "t_w1": w1_bf, "t_w2": w2_bf,
            "t_b1": w["b1"], "t_b2": w["b2"], "t_pw1": w["pw1"],
            "t_pw2": w["pw2"], "t_pb1": w["pb1"], "t_pb2": w["pb2"],
        }
        for q in range(nchunk):
            im[f"t_h0q{q}"] = h0q[q]
        in_maps.append(im)

    from concourse.bass_utils import run_bass_kernel_spmd
    trace = os.environ.get("GNN_TRACE", "0") == "1"
    res = run_bass_kernel_spmd(nc, in_maps, core_ids=list(range(NCORES)),
                               trace=trace)
    if trace:
        kernel.last_results = res
        if os.environ.get("GNN_TRACE_QUIET", "0") != "1":
            print(f"HW exec time: {res.exec_time_ns} ns")

    gs = geom["gs"]
    out = np.zeros((G, EMB), np.float32)
    for c in range(NCORES):
        outT = res.results[c]["o_outT"]  # [emb, graph slots]
        ng = per_core[c]["ng"]
        out[gs[c]:gs[c] + ng] = outT[:, :ng].T
    return out


# revision 19
# speedup vs baseline: 1.2718x; 1.2718x over previous
"""GIN message-passing network on 8 Trainium2 NeuronCores (Bass/Tile).

Strategy:
  - Nodes are split into 8 contiguous ranges at graph boundaries (so mean/sum
    pooling is core-local). Edges are owned by the core owning their dst node.
  - Each core keeps a full copy of node features h in HBM for gathering,
    split into NCHUNK row-chunk tables (chunk q = blocks cb[q]..cb[q+1] of
    every owner). After each conv, NCHUNK chunked AllGathers rebuild the
    tables; chunk q's collective fires as soon as chunk q's new features are
    written, and the next conv's phase-q gathers wait only on chunk q —
    overlapping the collective with the previous/next conv's gather drain.
  - Aggregation (segment-sum over incoming edges): gpsimd dma_gather pulls
    h[src] rows in fixed 1024-index calls rotating over the 4 SWDGE queues
    (single_packet=True: descriptors bundle into packets, ~15% faster drain).
    Edges are sorted by (src chunk phase, dst block); indices are int16
    relative to a per-chunk mid-table base. Preprocessing guarantees every
    call ends on a non-negative index (the gather ucode faults on trailing
    negatives). A DVE is_equal against an iota row builds each chunk's
    [128 edge x 128 node] one-hot, and PE matmuls Mg.T @ onehot accumulate
    agg^T in PSUM per 128-node dst block; later phases DVE-add into zT.
  - The GIN MLP runs transposed (features on partitions) in bf16 so
    biases+ReLU fuse as per-partition scalar.activation; a PE transpose
    yields node-major h_new for the next round's gather table.
  - Pooling: one-hot graph matmul accumulated over all blocks, then the
    post-MLP, all on-device; host reassembles the [256, 128] output.
"""

import os
import numpy as np
import ml_dtypes

N = 50000
E = 800000
NF = 9
EMB = 128
HID = 256
L = 3
NUM_CONVS = 2
G = 256
NCORES = 8
P = 128


def _preprocess(x, edge_index, batch, nchunk, mlpg, gch):
    """Host-side graph partitioning and phase-chunked edge layout."""
    gstart = np.searchsorted(batch, np.arange(G + 1))  # [G+1]

    # core graph splits balancing node counts
    gs = [0]
    for c in range(1, NCORES):
        t = (c * N) // NCORES
        i = int(np.searchsorted(gstart, t))
        if i > 0 and (i >= G + 1 or abs(int(gstart[i - 1]) - t) <= abs(int(gstart[i]) - t)):
            i -= 1
        i = max(gs[-1] + 1, min(i, G - (NCORES - c)))
        gs.append(i)
    gs.append(G)
    gs = np.array(gs, np.int64)
    ns = gstart[gs]  # node split points, ns[0]=0, ns[8]=N

    ncounts = np.diff(ns)
    NPAD = int(-(-ncounts.max() // P) * P)
    NB = NPAD // P

    # chunk boundaries in blocks, group-aligned. GNN_CB overrides cb[1] for
    # the asymmetric big-chunk/early-collective split.
    cb = [0]
    cbenv = os.environ.get("GNN_CB")
    if nchunk == 2 and cbenv:
        cb.append(min(((int(cbenv) // mlpg) * mlpg) or mlpg, NB - 1))
    else:
        for q in range(1, nchunk):
            v = ((q * NB) // nchunk // mlpg) * mlpg
            v = max(v, cb[-1] + mlpg)
            cb.append(min(v, NB))
    cb.append(NB)
    cb = np.array(cb, np.int64)
    CR = np.diff(cb) * P            # rows per chunk (per owner)
    ROWS_q = NCORES * CR            # rows per chunk table
    BASE_q = ROWS_q // 2

    chunk_of_block = np.searchsorted(cb, np.arange(NB), side="right") - 1

    node_ids = np.arange(N, dtype=np.int64)
    node_owner = np.searchsorted(ns, node_ids, side="right") - 1
    loc = node_ids - ns[node_owner]
    blkl = loc >> 7
    node_q = chunk_of_block[blkl]
    node_row = node_owner * CR[node_q] + (loc - cb[node_q] * P)

    src = np.asarray(edge_index[0], np.int64)
    dst = np.asarray(edge_index[1], np.int64)
    src_q = node_q[src]
    src_row = node_row[src]
    dst_owner = node_owner[dst]
    dl_all = dst - ns[dst_owner]
    blk_all = dl_all >> 7

    # per-(core, phase, block) counts -> shared chunk counts K[q][b]
    cnt = np.zeros((NCORES, nchunk, NB), np.int64)
    np.add.at(cnt, (dst_owner, src_q, blk_all), 1)
    K_qb = (-(-cnt.max(axis=0) // P)).astype(np.int64)  # [nchunk, NB]
    # every block needs at least one chunk overall (pads handle empties)
    empt = K_qb.sum(axis=0) == 0
    K_qb[0][empt] = 1
    # global chunk offsets: phases concatenated
    o_flat = np.concatenate([[0], np.cumsum(K_qb.reshape(-1))])
    o_qb = o_flat[:-1].reshape(nchunk, NB)       # [nchunk, NB]
    CHT_q = K_qb.sum(axis=1)                      # chunks per phase
    pstart = np.concatenate([[0], np.cumsum(CHT_q)])  # phase chunk offsets
    CHT = int(pstart[-1])
    NIDX = CHT * P

    # fixed-size gather calls per phase: (global chunk start, n chunks)
    calls = []   # list per phase
    for q in range(nchunk):
        cl = []
        c0 = int(pstart[q])
        while c0 < pstart[q + 1]:
            cn = int(min(gch, pstart[q + 1] - c0))
            cl.append((c0, cn))
            c0 += cn
        calls.append(cl)

    blk_of_chunk = np.empty(CHT, np.int64)
    for q in range(nchunk):
        blk_of_chunk[pstart[q]:pstart[q + 1]] = np.repeat(np.arange(NB), K_qb[q])

    per_core = []
    for c in range(NCORES):
        m = dst_owner == c
        sq = src_q[m]
        srow = src_row[m]
        rel = srow - BASE_q[sq]
        dl = dl_all[m]
        blk = blk_all[m]
        din = dl & 127
        order = np.lexsort(((rel >= 0), blk, sq))  # phase, block, neg-rel first
        rel, din, blk, sq = rel[order], din[order], blk[order], sq[order]

        # rank within (phase, block) segment
        seg = sq * NB + blk
        ccnt = np.bincount(seg, minlength=nchunk * NB)
        first = np.concatenate([[0], np.cumsum(ccnt)])[:-1]
        rank = np.arange(len(rel)) - first[seg]
        pos = o_qb[sq, blk] * P + rank

        flat_rel = np.zeros(NIDX, np.int32)  # pads: rel=0 (row BASE, valid)
        flat_rel[pos] = rel
        flat_din = np.full(NIDX, -1.0, np.float32)
        flat_din[pos] = din.astype(np.float32)
        # trailing-trim guard: each gather CALL must end on a non-negative
        # index (the HW chokes on trailing negatives). Swap the call-end slot
        # with a non-negative slot of the same (phase, block) segment.
        call_ends = set()
        for q in range(nchunk):
            for (c0, cn) in calls[q]:
                call_ends.add((c0 + cn) * P - 1)
        for end in sorted(call_ends):
            if flat_rel[end] >= 0:
                continue
            ch = end // P
            b = int(blk_of_chunk[ch])
            q = int(np.searchsorted(pstart, ch, side="right") - 1)
            seg0 = int(o_qb[q, b]) * P
            seg1 = seg0 + int(K_qb[q, b]) * P
            cand = np.nonzero(flat_rel[seg0:seg1] >= 0)[0]
            cand = [seg0 + int(j) for j in cand
                    if (seg0 + int(j)) not in call_ends]
            assert cand, "no non-negative slot available in segment"
            j = cand[-1]
            flat_rel[[j, end]] = flat_rel[[end, j]]
            flat_din[[j, end]] = flat_din[[end, j]]

        assert flat_rel.min() >= -32768 and flat_rel.max() < 32768
        idx16 = flat_rel.astype(np.int16).reshape(-1, 16).T  # [16, NIDX/16]
        idx_np = np.tile(idx16, (8, 1)).copy()  # [128, NIDX/16]
        dstloc_np = flat_din.reshape(CHT, P).T.copy()  # [128, CHT]

        # pooling one-hot + inverse counts
        ng = int(gs[c + 1] - gs[c])
        assert ng <= P
        bl = batch[ns[c]:ns[c + 1]] - gs[c]
        n_c = int(ncounts[c])
        ohg = np.zeros((NPAD, P), np.float32)
        ohg[np.arange(n_c), bl] = 1.0
        ohg_t = ohg.reshape(NB, P, P).transpose(1, 0, 2).reshape(P, NB * P).astype(ml_dtypes.bfloat16)
        cnts = np.bincount(bl, minlength=P)[:P]
        invc = np.zeros((P, 1), np.float32)
        invc[:ng, 0] = 1.0 / np.maximum(cnts[:ng], 1)

        per_core.append(dict(idx=idx_np, dstloc=dstloc_np, ohg=ohg_t, invc=invc,
                             ng=ng, n_c=n_c))

    # initial h chunk tables (bf16, same layout as the allgather outputs)
    h0q = []
    for q in range(nchunk):
        t = np.zeros((int(ROWS_q[q]), EMB), np.float32)
        for c in range(NCORES):
            r0 = int(cb[q]) * P
            r1 = min(int(cb[q + 1]) * P, int(ncounts[c]))
            if r1 > r0:
                t[c * int(CR[q]):c * int(CR[q]) + (r1 - r0), :NF] = \
                    x[ns[c] + r0:ns[c] + r1]
        h0q.append(t.astype(ml_dtypes.bfloat16))

    geom = dict(NPAD=NPAD, NB=NB, cb=cb, CR=CR, ROWS_q=ROWS_q, BASE_q=BASE_q,
                K_qb=K_qb, o_qb=o_qb, CHT=CHT, NIDX=NIDX, pstart=pstart,
                calls=calls, ns=ns, gs=gs, nchunk=nchunk)
    return geom, per_core, h0q


def _pack_weights(gin_w1, gin_b1, gin_w2, gin_b2, post_w1, post_b1, post_w2,
                  post_b2):
    w1 = np.concatenate([gin_w1[l] for l in range(L)], axis=1)  # [128, 768]
    w2 = np.concatenate(
        [gin_w2[l][h * P:(h + 1) * P, :] for l in range(L) for h in (0, 1)],
        axis=1)  # [128, 768]
    b1 = np.stack([gin_b1[l][h * P:(h + 1) * P] for l in range(L) for h in (0, 1)],
                  axis=1)  # [128, 6]
    b2 = np.stack([gin_b2[l] for l in range(L)], axis=1)  # [128, 3]
    pw1 = np.concatenate(
        [post_w1[kc * P:(kc + 1) * P, mh * P:(mh + 1) * P]
         for kc in (0, 1) for mh in (0, 1)], axis=1)  # [128, 512]
    pw2 = np.concatenate([post_w2[kc * P:(kc + 1) * P, :] for kc in (0, 1)],
                         axis=1)  # [128, 256]
    pb1 = np.stack([post_b1[mh * P:(mh + 1) * P] for mh in (0, 1)], axis=1)
    pb2 = post_b2[:, None]
    return dict(w1=w1, w2=w2, b1=b1, b2=b2, pw1=pw1, pw2=pw2, pb1=pb1, pb2=pb2)


def _build_program(geom, n_convs, reps=1):
    import concourse.bass as bass
    import concourse.bacc as bacc
    import concourse.tile as tile
    import concourse.mybir as mybir
    from concourse.masks import make_identity

    F32 = mybir.dt.float32
    BF16 = mybir.dt.bfloat16
    I16 = mybir.dt.int16
    Relu = mybir.ActivationFunctionType.Relu

    NPAD, NB = geom["NPAD"], geom["NB"]
    cb, CR, ROWS_q, BASE_q = geom["cb"], geom["CR"], geom["ROWS_q"], geom["BASE_q"]
    K_qb, o_qb, CHT, NIDX = geom["K_qb"], geom["o_qb"], geom["CHT"], geom["NIDX"]
    calls, pstart = geom["calls"], geom["pstart"]
    nchunk = geom["nchunk"]

    n_queues = int(os.environ.get("GNN_GQ", "4"))
    OHG = int(os.environ.get("GNN_OHG", "4"))  # onehot chunks per DVE op
    MLPG = 4     # 128-node blocks per MLP group (moving dim 512)
    GCH = int(os.environ.get("GNN_GCH", "8"))  # chunks per gather call
    single_packet = os.environ.get("GNN_SP", "1") == "1"
    cc_delay = int(os.environ.get("GNN_CCDELAY", "1"))
    # calls per conv pre-generated (prepare_only) during the previous conv's
    # collective stall; fired by trigger_dma once the table lands
    prep_n = int(os.environ.get("GNN_PREP", "0"))

    ndev = int(os.environ.get("GNN_NDEV", str(NCORES)))
    no_cc = os.environ.get("GNN_NO_CC", "0") == "1"
    nc = bacc.Bacc("TRN2", target_bir_lowering=False, debug=False,
                   enable_asserts=True, num_devices=ndev,
                   num_swdge_queues=4,
                   dynamic_dma_scratch_size=int(os.environ.get(
                       "GNN_DMA_SCRATCH", "98304")))

    t_h0q = [nc.dram_tensor(f"t_h0q{q}", [int(ROWS_q[q]), EMB], BF16,
                            kind="ExternalInput") for q in range(nchunk)]
    t_h0T = nc.dram_tensor("t_h0T", [P, NPAD], BF16, kind="ExternalInput")
    t_idx = nc.dram_tensor("t_idx", [P, NIDX // 16], I16, kind="ExternalInput")
    t_dstloc = nc.dram_tensor("t_dstloc", [P, CHT], F32, kind="ExternalInput")
    t_iota = nc.dram_tensor("t_iota", [P, OHG * P], F32, kind="ExternalInput")
    t_ohg = nc.dram_tensor("t_ohg", [P, NB * P], BF16, kind="ExternalInput")
    t_invc = nc.dram_tensor("t_invc", [P, 1], F32, kind="ExternalInput")
    t_w1 = nc.dram_tensor("t_w1", [P, L * 2 * P], BF16, kind="ExternalInput")
    t_w2 = nc.dram_tensor("t_w2", [P, L * 2 * P], BF16, kind="ExternalInput")
    t_b1 = nc.dram_tensor("t_b1", [P, L * 2], F32, kind="ExternalInput")
    t_b2 = nc.dram_tensor("t_b2", [P, L], F32, kind="ExternalInput")
    t_pw1 = nc.dram_tensor("t_pw1", [P, 4 * P], F32, kind="ExternalInput")
    t_pw2 = nc.dram_tensor("t_pw2", [P, 2 * P], F32, kind="ExternalInput")
    t_pb1 = nc.dram_tensor("t_pb1", [P, 2], F32, kind="ExternalInput")
    t_pb2 = nc.dram_tensor("t_pb2", [P, 1], F32, kind="ExternalInput")
    o_outT = nc.dram_tensor("o_outT", [P, P], F32, kind="ExternalOutput")

    # MLP block groups
    groups = []
    b0 = 0
    while b0 < NB:
        groups.append((b0, min(b0 + MLPG, NB)))
        b0 += MLPG

    qsems = [nc.alloc_semaphore(f"gsem{q}") for q in range(n_queues)]

    with tile.TileContext(nc) as tc:
        with tc.tile_pool(name="const", bufs=1) as cp, \
             tc.tile_pool(name="mgp", bufs=int(os.environ.get("GNN_MGB", str(12 + prep_n)))) as mgp, \
             tc.tile_pool(name="work", bufs=2) as wp, \
             tc.tile_pool(name="oh", bufs=4) as ohp, \
             tc.tile_pool(name="psA", bufs=2, space="PSUM") as psA, \
             tc.tile_pool(name="psT", bufs=1, space="PSUM") as psT_pool, \
             tc.tile_pool(name="psB", bufs=2, space="PSUM") as psB, \
             tc.tile_pool(name="psM", bufs=1, space="PSUM") as psM, \
             tc.tile_pool(name="psC", bufs=1, space="PSUM") as psC, \
             tc.tile_pool(name="dram", bufs=1, space="DRAM") as dram:

            idx_sb = cp.tile([P, NIDX // 16], I16)
            dstloc_sb = cp.tile([P, CHT], F32)
            iota_sb = cp.tile([P, OHG * P], F32)
            ohg_sb = cp.tile([P, NB * P], BF16)
            invc_sb = cp.tile([P, 1], F32)
            w1_sb = cp.tile([P, L * 2 * P], BF16)
            w2_sb = cp.tile([P, L * 2 * P], BF16)
            b1_sb = cp.tile([P, L * 2], F32)
            b2_sb = cp.tile([P, L], F32)
            pw1_sb = cp.tile([P, 4 * P], F32)
            pw2_sb = cp.tile([P, 2 * P], F32)
            pb1_sb = cp.tile([P, 2], F32)
            pb2_sb = cp.tile([P, 1], F32)
            ident = cp.tile([P, P], F32)
            ident_bf = cp.tile([P, P], BF16)
            for sb_t, dr_t in [(idx_sb, t_idx), (dstloc_sb, t_dstloc),
                               (iota_sb, t_iota), (ohg_sb, t_ohg),
                               (invc_sb, t_invc), (w1_sb, t_w1), (w2_sb, t_w2),
                               (b1_sb, t_b1), (b2_sb, t_b2), (pw1_sb, t_pw1),
                               (pw2_sb, t_pw2), (pb1_sb, t_pb1),
                               (pb2_sb, t_pb2)]:
                nc.sync.dma_start(sb_t[:], dr_t[:])
            make_identity(nc, ident[:])
            make_identity(nc, ident_bf[:])

            # persistent feature-major h (ping-pong) + zT staging
            hT0 = cp.tile([P, NPAD], BF16)
            hT1 = cp.tile([P, NPAD], BF16)
            hT_pp = [hT0, hT1]
            zT_all = cp.tile([P, NPAD], BF16)
            nc.sync.dma_start(hT0[:], t_h0T[:])

            hnew0 = dram.tile([NPAD, EMB], BF16)
            hnew1 = dram.tile([NPAD, EMB], BF16)
            hnew_pp = [hnew0, hnew1]
            n_cc = max(reps * n_convs - 1, 1)
            hfq_cv = [[dram.tile([int(ROWS_q[q]), EMB], BF16,
                                 addr_space="Shared", name=f"hf{i}q{q}")
                       for q in range(nchunk)] for i in range(n_cc)]

            psum_pool = psC.tile([P, P], F32, space="PSUM", tag="pool")

            def src_tab_for(gc):
                return [t_h0q[q] if (gc == 0 or no_cc) else
                        hfq_cv[gc - 1][q] for q in range(nchunk)]

            # per-conv gather-call state (survives across the conv loop so a
            # boundary can pre-generate the next conv's descriptors)
            states = {}

            def get_state(gc):
                if gc not in states:
                    states[gc] = dict(next_call=[0] * nchunk, chunk2mg={},
                                      ncalls=0, trig=set())
                return states[gc]

            nidx_regs = {}  # hoisted num_idxs registers (one MOVE per size)

            def issue_one_call(gc, q, prep):
                st = get_state(gc)
                c0, cn = calls[q][st["next_call"][q]]
                mg = mgp.tile([P, GCH, P], BF16, tag="mg",
                              name=f"mg_{gc}_{q}_{c0}")
                qn = st["ncalls"] % n_queues
                kw = dict(prepare_only=True, sem=qsems[qn]) if prep else {}
                if cn not in nidx_regs:
                    nidx_regs[cn] = nc.gpsimd.to_reg(cn * P)
                nc.gpsimd.dma_gather(
                    out_ap=mg[:, :cn, :],
                    in_ap=src_tab_for(gc)[q][int(BASE_q[q]):, :],
                    idxs_ap=idx_sb[:, c0 * 8:(c0 + cn) * 8],
                    num_idxs=cn * P,
                    num_idxs_reg=nidx_regs[cn],
                    elem_size=EMB,
                    single_packet=single_packet,
                    queue_num=qn,
                    **kw,
                )
                if prep:
                    st["trig"].add(qn)
                st["ncalls"] += 1
                for j in range(cn):
                    st["chunk2mg"][c0 + j] = (mg, j)
                st["next_call"][q] += 1

            for gc in range(reps * n_convs):
                r, c = divmod(gc, n_convs)
                l = min(c // NUM_CONVS, L - 1)
                hT_cur = hT_pp[gc % 2]
                hT_nxt = hT_pp[(gc + 1) % 2]
                last = gc == reps * n_convs - 1

                st_gc = get_state(gc)
                # fire any descriptors pre-generated at the previous boundary
                for qn in sorted(st_gc["trig"]):
                    nc.gpsimd.trigger_dma(count=None, queue_num=qn)
                st_gc["trig"].clear()
                chunk2mg = st_gc["chunk2mg"]

                def issue_calls(q, need_end):
                    st = st_gc
                    while (st["next_call"][q] < len(calls[q])
                           and calls[q][st["next_call"][q]][0] < need_end):
                        issue_one_call(gc, q, False)

                def agg_block(q, b, first):
                    # psum-accumulate phase q of block b, then fold into zT
                    kb = int(K_qb[q][b])
                    bs = slice(b * P, (b + 1) * P)
                    if kb == 0:
                        if first:
                            nc.vector.tensor_copy(out=zT_all[:, bs],
                                                  in_=hT_cur[:, bs])
                        return
                    ob = int(o_qb[q][b])
                    issue_calls(q, ob + kb)
                    psumA = psA.tile([P, P], F32, space="PSUM", tag="agg",
                                     name=f"agg_{gc}_{q}_{b}")
                    n_oh = (kb + OHG - 1) // OHG
                    ohts = []
                    for j in range(n_oh):
                        k0 = j * OHG
                        kn = min(OHG, kb - k0)
                        oht = ohp.tile([P, OHG, P], BF16, tag="oh",
                                       name=f"oh_{gc}_{q}_{b}_{j}")
                        nc.vector.tensor_tensor(
                            out=oht[:, :kn, :],
                            in0=iota_sb[:, :kn * P].rearrange(
                                "p (a b) -> p a b", b=P),
                            in1=dstloc_sb[:, ob + k0:ob + k0 + kn]
                                .to_broadcast([P, kn, P]),
                            op=mybir.AluOpType.is_equal)
                        ohts.append((oht, k0, kn))
                    for oht, k0, kn in ohts:
                        for kk in range(kn):
                            k = k0 + kk
                            mg, off = chunk2mg[ob + k]
                            nc.tensor.matmul(out=psumA[:],
                                             lhsT=mg[:, off, :],
                                             rhs=oht[:, kk, :],
                                             start=(k == 0),
                                             stop=(k == kb - 1))
                    nc.vector.tensor_add(
                        out=zT_all[:, bs], in0=psumA[:],
                        in1=hT_cur[:, bs] if first else zT_all[:, bs])

                # small trailing phases first (their tables landed early via
                # the tiny tail collectives); phase 0 fused with the MLP
                for q in range(1, nchunk):
                    for b in range(NB):
                        agg_block(q, b, q == 1)

                pending_cc = []  # (chunk idx, groups countdown)
                for gi, (g0, g1) in enumerate(groups):
                    for b in range(g0, g1):
                        agg_block(0, b, nchunk == 1)
                    # grouped MLP: moving dim = 128 * (g1 - g0)
                    gw = (g1 - g0) * P
                    gsl = slice(g0 * P, g0 * P + gw)
                    z1 = []
                    for mh in range(2):
                        ps1 = psB.tile([P, 512], F32, space="PSUM", tag="mm1",
                                       name=f"mm1_{gc}_{g0}_{mh}")
                        nc.tensor.matmul(
                            out=ps1[:, :gw],
                            lhsT=w1_sb[:, (l * 2 + mh) * P:(l * 2 + mh + 1) * P],
                            rhs=zT_all[:, gsl], start=True, stop=True)
                        z1t = wp.tile([P, 512], BF16, tag=f"z1_{mh}",
                                      name=f"z1_{gc}_{g0}_{mh}")
                        nc.scalar.activation(
                            out=z1t[:, :gw], in_=ps1[:, :gw], func=Relu,
                            bias=b1_sb[:, l * 2 + mh:l * 2 + mh + 1])
                        z1.append(z1t)
                    ps2 = psM.tile([P, 512], F32, space="PSUM", tag="mm2",
                                   name=f"mm2_{gc}_{g0}")
                    for mh in range(2):
                        nc.tensor.matmul(
                            out=ps2[:, :gw],
                            lhsT=w2_sb[:, (l * 2 + mh) * P:(l * 2 + mh + 1) * P],
                            rhs=z1[mh][:, :gw], start=(mh == 0), stop=(mh == 1))
                    nc.scalar.activation(out=hT_nxt[:, gsl], in_=ps2[:, :gw],
                                         func=Relu, bias=b2_sb[:, l:l + 1])
                    # node-major h_new per block (for allgather / pooling)
                    for b in range(g0, g1):
                        bs = slice(b * P, (b + 1) * P)
                        psT = psT_pool.tile([P, P], BF16, space="PSUM", tag="tp",
                                            name=f"tp_{gc}_{b}")
                        nc.tensor.transpose(out=psT[:], in_=hT_nxt[:, bs],
                                            identity=ident_bf[:])
                        hnode = wp.tile([P, P], BF16, tag="hnode",
                                        name=f"hn_{gc}_{b}")
                        nc.scalar.copy(out=hnode[:], in_=psT[:])
                        if not last:
                            nc.sync.dma_start(hnew_pp[gc % 2][bs, :], hnode[:])
                        else:
                            nc.tensor.matmul(out=psum_pool[:],
                                             lhsT=ohg_sb[:, bs], rhs=hnode[:],
                                             start=(b == 0), stop=(b == NB - 1),
                                             skip_group_check=True)
                    # chunked allgathers, delayed a few groups so the hnew
                    # writes they wait on have drained (Pool queue-head stall)
                    if not last and not no_cc:
                        for q in range(nchunk):
                            if cb[q + 1] == g1:
                                pending_cc.append([q, cc_delay])
                        for pc in pending_cc:
                            pc[1] -= 1
                        while pending_cc and (pending_cc[0][1] < 0
                                              or gi == len(groups) - 1):
                            q = pending_cc.pop(0)[0]
                            nc.gpsimd.collective_compute(
                                "AllGather", mybir.AluOpType.bypass,
                                replica_groups=[list(range(NCORES))],
                                ins=[hnew_pp[gc % 2][int(cb[q]) * P:
                                                     int(cb[q + 1]) * P, :].opt()],
                                outs=[hfq_cv[gc][q].opt()])
                # pre-generate the next conv's first descriptors while the
                # collective runs (descriptor gen reads only idx metadata; the
                # deferred table read lands on trigger_dma)
                if not last:
                    st_nx = get_state(gc + 1)
                    for _ in range(prep_n):
                        if st_nx["next_call"][0] >= len(calls[0]):
                            break
                        issue_one_call(gc + 1, 0, True)

            # pooling epilogue
            sums_sb = cp.tile([P, P], F32)
            means_sb = cp.tile([P, P], F32)
            nc.vector.tensor_copy(out=sums_sb[:], in_=psum_pool[:])
            nc.vector.tensor_scalar(out=means_sb[:], in0=psum_pool[:],
                                    scalar1=invc_sb[:, 0:1], scalar2=None,
                                    op0=mybir.AluOpType.mult)
            psTs = psT_pool.tile([P, P], F32, space="PSUM", tag="tp")
            nc.tensor.transpose(out=psTs[:], in_=sums_sb[:], identity=ident[:])
            sT = cp.tile([P, P], F32)
            nc.scalar.copy(out=sT[:], in_=psTs[:])
            psTm = psT_pool.tile([P, P], F32, space="PSUM", tag="tp")
            nc.tensor.transpose(out=psTm[:], in_=means_sb[:], identity=ident[:])
            mT = cp.tile([P, P], F32)
            nc.scalar.copy(out=mT[:], in_=psTm[:])

            z1p = []
            for mh in range(2):
                ps3 = psB.tile([P, 512], F32, space="PSUM", tag="mm1")
                nc.tensor.matmul(out=ps3[:, :P],
                                 lhsT=pw1_sb[:, (0 * 2 + mh) * P:(0 * 2 + mh + 1) * P],
                                 rhs=sT[:], start=True, stop=False)
                nc.tensor.matmul(out=ps3[:, :P],
                                 lhsT=pw1_sb[:, (1 * 2 + mh) * P:(1 * 2 + mh + 1) * P],
                                 rhs=mT[:], start=False, stop=True)
                z1t = cp.tile([P, P], F32, name=f"z1p_{mh}")
                nc.scalar.activation(out=z1t[:], in_=ps3[:, :P], func=Relu,
                                     bias=pb1_sb[:, mh:mh + 1])
                z1p.append(z1t)
            ps4 = psM.tile([P, 512], F32, space="PSUM", tag="mm2")
            for kc in range(2):
                nc.tensor.matmul(out=ps4[:, :P], lhsT=pw2_sb[:, kc * P:(kc + 1) * P],
                                 rhs=z1p[kc][:], start=(kc == 0), stop=(kc == 1))
            out_sb = cp.tile([P, P], F32)
            nc.vector.tensor_scalar(out=out_sb[:], in0=ps4[:, :P],
                                    scalar1=pb2_sb[:, 0:1], scalar2=None,
                                    op0=mybir.AluOpType.add)
            nc.sync.dma_start(o_outT[:], out_sb[:])

    nc.compile()
    return nc


def kernel(**inputs):
    x = np.asarray(inputs["x"], np.float32)
    edge_index = np.asarray(inputs["edge_index"], np.int64)
    batch = np.asarray(inputs["batch"], np.int64)
    gin_w1 = np.asarray(inputs["gin_w1"], np.float32)
    gin_b1 = np.asarray(inputs["gin_b1"], np.float32)
    gin_w2 = np.asarray(inputs["gin_w2"], np.float32)
    gin_b2 = np.asarray(inputs["gin_b2"], np.float32)
    post_w1 = np.asarray(inputs["post_w1"], np.float32)
    post_b1 = np.asarray(inputs["post_b1"], np.float32)
    post_w2 = np.asarray(inputs["post_w2"], np.float32)
    post_b2 = np.asarray(inputs["post_b2"], np.float32)

    nchunk = int(os.environ.get("GNN_NCHUNK", "1"))
    MLPG = 4
    GCH = int(os.environ.get("GNN_GCH", "8"))
    geom, per_core, h0q = _preprocess(x, edge_index, batch, nchunk, MLPG, GCH)
    w = _pack_weights(gin_w1, gin_b1, gin_w2, gin_b2, post_w1, post_b1,
                      post_w2, post_b2)

    n_convs = int(os.environ.get("GNN_CONVS", L * NUM_CONVS))
    nc = _build_program(geom, n_convs, reps=int(os.environ.get('GNN_REPS', '1')))

    NPAD = geom["NPAD"]
    ns = geom["ns"]
    iota_np = np.tile(np.arange(128, dtype=np.float32),
                      (128, int(os.environ.get("GNN_OHG", "4"))))
    w1_bf = w["w1"].astype(ml_dtypes.bfloat16)
    w2_bf = w["w2"].astype(ml_dtypes.bfloat16)
    in_maps = []
    for c in range(NCORES):
        pc = per_core[c]
        h0T = np.zeros((P, NPAD), np.float32)
        n_c = int(ns[c + 1] - ns[c])
        h0T[:NF, :n_c] = x[ns[c]:ns[c + 1]].T
        im = {
            "t_h0T": h0T.astype(ml_dtypes.bfloat16), "t_idx": pc["idx"],
            "t_dstloc": pc["dstloc"], "t_iota": iota_np, "t_ohg": pc["ohg"],
            "t_invc": pc["invc"], "t_w1": w1_bf, "t_w2": w2_bf,
            "t_b1": w["b1"], "t_b2": w["b2"], "t_pw1": w["pw1"],
            "t_pw2": w["pw2"], "t_pb1": w["pb1"], "t_pb2": w["pb2"],
        }
        for q in range(nchunk):
            im[f"t_h0q{q}"] = h0q[q]
        in_maps.append(im)

    from concourse.bass_utils import run_bass_kernel_spmd
    trace = os.environ.get("GNN_TRACE", "0") == "1"
    res = run_bass_kernel_spmd(nc, in_maps, core_ids=list(range(NCORES)),
                               trace=trace)
    if trace:
        kernel.last_results = res
        if os.environ.get("GNN_TRACE_QUIET", "0") != "1":
            print(f"HW exec time: {res.exec_time_ns} ns")

    gs = geom["gs"]
    out = np.zeros((G, EMB), np.float32)
    for c in range(NCORES):
        outT = res.results[c]["o_outT"]  # [emb, graph slots]
        ng = per_core[c]["ng"]
        out[gs[c]:gs[c] + ng] = outT[:, :ng].T
    return out


# revision 20
# speedup vs baseline: 1.3691x; 1.0765x over previous
"""GIN message-passing network on 8 Trainium2 NeuronCores (Bass/Tile).

Strategy:
  - Nodes are split into 8 contiguous ranges at graph boundaries (so mean/sum
    pooling is core-local). Edges are owned by the core owning their dst node.
  - Each core keeps a full copy of node features h in HBM for gathering,
    split into NCHUNK row-chunk tables (chunk q = blocks cb[q]..cb[q+1] of
    every owner). After each conv, NCHUNK chunked AllGathers rebuild the
    tables; chunk q's collective fires as soon as chunk q's new features are
    written, and the next conv's phase-q gathers wait only on chunk q —
    overlapping the collective with the previous/next conv's gather drain.
  - Aggregation (segment-sum over incoming edges): gpsimd dma_gather pulls
    h[src] rows in fixed 1024-index calls rotating over the 4 SWDGE queues
    (single_packet=True: descriptors bundle into packets, ~15% faster drain).
    Edges are sorted by (src chunk phase, dst block); indices are int16
    relative to a per-chunk mid-table base. Preprocessing guarantees every
    call ends on a non-negative index (the gather ucode faults on trailing
    negatives). A DVE is_equal against an iota row builds each chunk's
    [128 edge x 128 node] one-hot, and PE matmuls Mg.T @ onehot accumulate
    agg^T in PSUM per 128-node dst block; later phases DVE-add into zT.
  - The GIN MLP runs transposed (features on partitions) in bf16 so
    biases+ReLU fuse as per-partition scalar.activation; a PE transpose
    yields node-major h_new for the next round's gather table.
  - Pooling: one-hot graph matmul accumulated over all blocks, then the
    post-MLP, all on-device; host reassembles the [256, 128] output.
"""

import os
import numpy as np
import ml_dtypes

N = 50000
E = 800000
NF = 9
EMB = 128
HID = 256
L = 3
NUM_CONVS = 2
G = 256
NCORES = 8
P = 128


def _preprocess(x, edge_index, batch, nchunk, mlpg, gch):
    """Host-side graph partitioning and phase-chunked edge layout."""
    gstart = np.searchsorted(batch, np.arange(G + 1))  # [G+1]

    # core graph splits balancing node counts
    gs = [0]
    for c in range(1, NCORES):
        t = (c * N) // NCORES
        i = int(np.searchsorted(gstart, t))
        if i > 0 and (i >= G + 1 or abs(int(gstart[i - 1]) - t) <= abs(int(gstart[i]) - t)):
            i -= 1
        i = max(gs[-1] + 1, min(i, G - (NCORES - c)))
        gs.append(i)
    gs.append(G)
    gs = np.array(gs, np.int64)
    ns = gstart[gs]  # node split points, ns[0]=0, ns[8]=N

    ncounts = np.diff(ns)
    NPAD = int(-(-ncounts.max() // P) * P)
    NB = NPAD // P

    # chunk boundaries in blocks, group-aligned. GNN_CB overrides cb[1] for
    # the asymmetric big-chunk/early-collective split.
    cb = [0]
    cbenv = os.environ.get("GNN_CB")
    if nchunk == 2 and cbenv:
        cb.append(min(((int(cbenv) // mlpg) * mlpg) or mlpg, NB - 1))
    else:
        for q in range(1, nchunk):
            v = ((q * NB) // nchunk // mlpg) * mlpg
            v = max(v, cb[-1] + mlpg)
            cb.append(min(v, NB))
    cb.append(NB)
    cb = np.array(cb, np.int64)
    CR = np.diff(cb) * P            # rows per chunk (per owner)
    ROWS_q = NCORES * CR            # rows per chunk table
    BASE_q = ROWS_q // 2

    chunk_of_block = np.searchsorted(cb, np.arange(NB), side="right") - 1

    node_ids = np.arange(N, dtype=np.int64)
    node_owner = np.searchsorted(ns, node_ids, side="right") - 1
    loc = node_ids - ns[node_owner]
    blkl = loc >> 7
    node_q = chunk_of_block[blkl]
    node_row = node_owner * CR[node_q] + (loc - cb[node_q] * P)

    src = np.asarray(edge_index[0], np.int64)
    dst = np.asarray(edge_index[1], np.int64)
    src_q = node_q[src]
    src_row = node_row[src]
    dst_owner = node_owner[dst]
    dl_all = dst - ns[dst_owner]
    blk_all = dl_all >> 7

    # per-(core, phase, block) counts -> shared chunk counts K[q][b]
    cnt = np.zeros((NCORES, nchunk, NB), np.int64)
    np.add.at(cnt, (dst_owner, src_q, blk_all), 1)
    K_qb = (-(-cnt.max(axis=0) // P)).astype(np.int64)  # [nchunk, NB]
    # every block needs at least one chunk overall (pads handle empties)
    empt = K_qb.sum(axis=0) == 0
    K_qb[0][empt] = 1
    # global chunk offsets: phases concatenated
    o_flat = np.concatenate([[0], np.cumsum(K_qb.reshape(-1))])
    o_qb = o_flat[:-1].reshape(nchunk, NB)       # [nchunk, NB]
    CHT_q = K_qb.sum(axis=1)                      # chunks per phase
    pstart = np.concatenate([[0], np.cumsum(CHT_q)])  # phase chunk offsets
    CHT = int(pstart[-1])
    NIDX = CHT * P

    # fixed-size gather calls per phase: (global chunk start, n chunks)
    calls = []   # list per phase
    for q in range(nchunk):
        cl = []
        c0 = int(pstart[q])
        while c0 < pstart[q + 1]:
            cn = int(min(gch, pstart[q + 1] - c0))
            cl.append((c0, cn))
            c0 += cn
        calls.append(cl)

    blk_of_chunk = np.empty(CHT, np.int64)
    for q in range(nchunk):
        blk_of_chunk[pstart[q]:pstart[q + 1]] = np.repeat(np.arange(NB), K_qb[q])

    per_core = []
    for c in range(NCORES):
        m = dst_owner == c
        sq = src_q[m]
        srow = src_row[m]
        rel = srow - BASE_q[sq]
        dl = dl_all[m]
        blk = blk_all[m]
        din = dl & 127
        order = np.lexsort(((rel >= 0), blk, sq))  # phase, block, neg-rel first
        rel, din, blk, sq = rel[order], din[order], blk[order], sq[order]

        # rank within (phase, block) segment
        seg = sq * NB + blk
        ccnt = np.bincount(seg, minlength=nchunk * NB)
        first = np.concatenate([[0], np.cumsum(ccnt)])[:-1]
        rank = np.arange(len(rel)) - first[seg]
        pos = o_qb[sq, blk] * P + rank

        flat_rel = np.zeros(NIDX, np.int32)  # pads: rel=0 (row BASE, valid)
        flat_rel[pos] = rel
        flat_din = np.full(NIDX, -1.0, np.float32)
        flat_din[pos] = din.astype(np.float32)
        # trailing-trim guard: each gather CALL must end on a non-negative
        # index (the HW chokes on trailing negatives). Swap the call-end slot
        # with a non-negative slot of the same (phase, block) segment.
        call_ends = set()
        for q in range(nchunk):
            for (c0, cn) in calls[q]:
                call_ends.add((c0 + cn) * P - 1)
        for end in sorted(call_ends):
            if flat_rel[end] >= 0:
                continue
            ch = end // P
            b = int(blk_of_chunk[ch])
            q = int(np.searchsorted(pstart, ch, side="right") - 1)
            seg0 = int(o_qb[q, b]) * P
            seg1 = seg0 + int(K_qb[q, b]) * P
            cand = np.nonzero(flat_rel[seg0:seg1] >= 0)[0]
            cand = [seg0 + int(j) for j in cand
                    if (seg0 + int(j)) not in call_ends]
            assert cand, "no non-negative slot available in segment"
            j = cand[-1]
            flat_rel[[j, end]] = flat_rel[[end, j]]
            flat_din[[j, end]] = flat_din[[end, j]]

        assert flat_rel.min() >= -32768 and flat_rel.max() < 32768
        idx16 = flat_rel.astype(np.int16).reshape(-1, 16).T  # [16, NIDX/16]
        idx_np = np.tile(idx16, (8, 1)).copy()  # [128, NIDX/16]
        dstloc_np = flat_din.reshape(CHT, P).T.copy()  # [128, CHT]

        # pooling one-hot + inverse counts
        ng = int(gs[c + 1] - gs[c])
        assert ng <= P
        bl = batch[ns[c]:ns[c + 1]] - gs[c]
        n_c = int(ncounts[c])
        ohg = np.zeros((NPAD, P), np.float32)
        ohg[np.arange(n_c), bl] = 1.0
        ohg_t = ohg.reshape(NB, P, P).transpose(1, 0, 2).reshape(P, NB * P).astype(ml_dtypes.bfloat16)
        cnts = np.bincount(bl, minlength=P)[:P]
        invc = np.zeros((P, 1), np.float32)
        invc[:ng, 0] = 1.0 / np.maximum(cnts[:ng], 1)

        per_core.append(dict(idx=idx_np, dstloc=dstloc_np, ohg=ohg_t, invc=invc,
                             ng=ng, n_c=n_c))

    # initial h chunk tables (bf16, same layout as the allgather outputs)
    h0q = []
    for q in range(nchunk):
        t = np.zeros((int(ROWS_q[q]), EMB), np.float32)
        for c in range(NCORES):
            r0 = int(cb[q]) * P
            r1 = min(int(cb[q + 1]) * P, int(ncounts[c]))
            if r1 > r0:
                t[c * int(CR[q]):c * int(CR[q]) + (r1 - r0), :NF] = \
                    x[ns[c] + r0:ns[c] + r1]
        h0q.append(t.astype(ml_dtypes.bfloat16))

    geom = dict(NPAD=NPAD, NB=NB, cb=cb, CR=CR, ROWS_q=ROWS_q, BASE_q=BASE_q,
                K_qb=K_qb, o_qb=o_qb, CHT=CHT, NIDX=NIDX, pstart=pstart,
                calls=calls, ns=ns, gs=gs, nchunk=nchunk)
    return geom, per_core, h0q


def _pack_weights(gin_w1, gin_b1, gin_w2, gin_b2, post_w1, post_b1, post_w2,
                  post_b2):
    w1 = np.concatenate([gin_w1[l] for l in range(L)], axis=1)  # [128, 768]
    w2 = np.concatenate(
        [gin_w2[l][h * P:(h + 1) * P, :] for l in range(L) for h in (0, 1)],
        axis=1)  # [128, 768]
    b1 = np.stack([gin_b1[l][h * P:(h + 1) * P] for l in range(L) for h in (0, 1)],
                  axis=1)  # [128, 6]
    b2 = np.stack([gin_b2[l] for l in range(L)], axis=1)  # [128, 3]
    pw1 = np.concatenate(
        [post_w1[kc * P:(kc + 1) * P, mh * P:(mh + 1) * P]
         for kc in (0, 1) for mh in (0, 1)], axis=1)  # [128, 512]
    pw2 = np.concatenate([post_w2[kc * P:(kc + 1) * P, :] for kc in (0, 1)],
                         axis=1)  # [128, 256]
    pb1 = np.stack([post_b1[mh * P:(mh + 1) * P] for mh in (0, 1)], axis=1)
    pb2 = post_b2[:, None]
    return dict(w1=w1, w2=w2, b1=b1, b2=b2, pw1=pw1, pw2=pw2, pb1=pb1, pb2=pb2)


def _build_program(geom, n_convs, reps=1):
    import concourse.bass as bass
    import concourse.bacc as bacc
    import concourse.tile as tile
    import concourse.mybir as mybir
    from concourse.masks import make_identity

    F32 = mybir.dt.float32
    BF16 = mybir.dt.bfloat16
    I16 = mybir.dt.int16
    Relu = mybir.ActivationFunctionType.Relu

    NPAD, NB = geom["NPAD"], geom["NB"]
    cb, CR, ROWS_q, BASE_q = geom["cb"], geom["CR"], geom["ROWS_q"], geom["BASE_q"]
    K_qb, o_qb, CHT, NIDX = geom["K_qb"], geom["o_qb"], geom["CHT"], geom["NIDX"]
    calls, pstart = geom["calls"], geom["pstart"]
    nchunk = geom["nchunk"]

    n_queues = int(os.environ.get("GNN_GQ", "4"))
    OHG = int(os.environ.get("GNN_OHG", "4"))  # onehot chunks per DVE op
    MLPG = 4     # 128-node blocks per MLP group (moving dim 512)
    GCH = int(os.environ.get("GNN_GCH", "8"))  # chunks per gather call
    single_packet = os.environ.get("GNN_SP", "1") == "1"
    cc_delay = int(os.environ.get("GNN_CCDELAY", "1"))
    # calls per conv pre-generated (prepare_only) during the previous conv's
    # collective stall; fired by trigger_dma once the table lands
    prep_n = int(os.environ.get("GNN_PREP", "0"))

    ndev = int(os.environ.get("GNN_NDEV", str(NCORES)))
    no_cc = os.environ.get("GNN_NO_CC", "0") == "1"
    nc = bacc.Bacc("TRN2", target_bir_lowering=False, debug=False,
                   enable_asserts=True, num_devices=ndev,
                   num_swdge_queues=4,
                   dynamic_dma_scratch_size=int(os.environ.get(
                       "GNN_DMA_SCRATCH", "98304")))

    t_h0q = [nc.dram_tensor(f"t_h0q{q}", [int(ROWS_q[q]), EMB], BF16,
                            kind="ExternalInput") for q in range(nchunk)]
    t_h0T = nc.dram_tensor("t_h0T", [P, NPAD], BF16, kind="ExternalInput")
    t_idx = nc.dram_tensor("t_idx", [P, NIDX // 16], I16, kind="ExternalInput")
    t_dstloc = nc.dram_tensor("t_dstloc", [P, CHT], F32, kind="ExternalInput")
    t_iota = nc.dram_tensor("t_iota", [P, OHG * P], F32, kind="ExternalInput")
    t_ohg = nc.dram_tensor("t_ohg", [P, NB * P], BF16, kind="ExternalInput")
    t_invc = nc.dram_tensor("t_invc", [P, 1], F32, kind="ExternalInput")
    t_w1 = nc.dram_tensor("t_w1", [P, L * 2 * P], BF16, kind="ExternalInput")
    t_w2 = nc.dram_tensor("t_w2", [P, L * 2 * P], BF16, kind="ExternalInput")
    t_b1 = nc.dram_tensor("t_b1", [P, L * 2], F32, kind="ExternalInput")
    t_b2 = nc.dram_tensor("t_b2", [P, L], F32, kind="ExternalInput")
    t_pw1 = nc.dram_tensor("t_pw1", [P, 4 * P], F32, kind="ExternalInput")
    t_pw2 = nc.dram_tensor("t_pw2", [P, 2 * P], F32, kind="ExternalInput")
    t_pb1 = nc.dram_tensor("t_pb1", [P, 2], F32, kind="ExternalInput")
    t_pb2 = nc.dram_tensor("t_pb2", [P, 1], F32, kind="ExternalInput")
    o_outT = nc.dram_tensor("o_outT", [P, P], F32, kind="ExternalOutput")

    # MLP block groups
    groups = []
    b0 = 0
    while b0 < NB:
        groups.append((b0, min(b0 + MLPG, NB)))
        b0 += MLPG

    qsems = [nc.alloc_semaphore(f"gsem{q}") for q in range(n_queues)]

    with tile.TileContext(nc) as tc:
        with tc.tile_pool(name="const", bufs=1) as cp, \
             tc.tile_pool(name="mgp", bufs=int(os.environ.get("GNN_MGB", str(12 + prep_n)))) as mgp, \
             tc.tile_pool(name="work", bufs=2) as wp, \
             tc.tile_pool(name="oh", bufs=4) as ohp, \
             tc.tile_pool(name="psA", bufs=2, space="PSUM") as psA, \
             tc.tile_pool(name="psT", bufs=1, space="PSUM") as psT_pool, \
             tc.tile_pool(name="psB", bufs=2, space="PSUM") as psB, \
             tc.tile_pool(name="psM", bufs=1, space="PSUM") as psM, \
             tc.tile_pool(name="psC", bufs=1, space="PSUM") as psC, \
             tc.tile_pool(name="dram", bufs=1, space="DRAM") as dram:

            idx_sb = cp.tile([P, NIDX // 16], I16)
            dstloc_sb = cp.tile([P, CHT], F32)
            iota_sb = cp.tile([P, OHG * P], F32)
            ohg_sb = cp.tile([P, NB * P], BF16)
            invc_sb = cp.tile([P, 1], F32)
            w1_sb = cp.tile([P, L * 2 * P], BF16)
            w2_sb = cp.tile([P, L * 2 * P], BF16)
            b1_sb = cp.tile([P, L * 2], F32)
            b2_sb = cp.tile([P, L], F32)
            pw1_sb = cp.tile([P, 4 * P], F32)
            pw2_sb = cp.tile([P, 2 * P], F32)
            pb1_sb = cp.tile([P, 2], F32)
            pb2_sb = cp.tile([P, 1], F32)
            ident = cp.tile([P, P], F32)
            ident_bf = cp.tile([P, P], BF16)
            for sb_t, dr_t in [(idx_sb, t_idx), (dstloc_sb, t_dstloc),
                               (iota_sb, t_iota), (ohg_sb, t_ohg),
                               (invc_sb, t_invc), (w1_sb, t_w1), (w2_sb, t_w2),
                               (b1_sb, t_b1), (b2_sb, t_b2), (pw1_sb, t_pw1),
                               (pw2_sb, t_pw2), (pb1_sb, t_pb1),
                               (pb2_sb, t_pb2)]:
                nc.sync.dma_start(sb_t[:], dr_t[:])
            make_identity(nc, ident[:])
            make_identity(nc, ident_bf[:])

            # persistent feature-major h (ping-pong) + zT staging
            hT0 = cp.tile([P, NPAD], BF16)
            hT1 = cp.tile([P, NPAD], BF16)
            hT_pp = [hT0, hT1]
            zT_all = cp.tile([P, NPAD], BF16)
            nc.sync.dma_start(hT0[:], t_h0T[:])

            hnew0 = dram.tile([NPAD, EMB], BF16)
            hnew1 = dram.tile([NPAD, EMB], BF16)
            hnew_pp = [hnew0, hnew1]
            n_cc = max(reps * n_convs - 1, 1)
            hfq_cv = [[dram.tile([int(ROWS_q[q]), EMB], BF16,
                                 addr_space="Shared", name=f"hf{i}q{q}")
                       for q in range(nchunk)] for i in range(n_cc)]

            psum_pool = psC.tile([P, P], F32, space="PSUM", tag="pool")

            def src_tab_for(gc):
                return [t_h0q[q] if (gc == 0 or no_cc) else
                        hfq_cv[gc - 1][q] for q in range(nchunk)]

            # per-conv gather-call state (survives across the conv loop so a
            # boundary can pre-generate the next conv's descriptors)
            states = {}

            def get_state(gc):
                if gc not in states:
                    states[gc] = dict(next_call=[0] * nchunk, chunk2mg={},
                                      ncalls=0, trig=set())
                return states[gc]

            def issue_one_call(gc, q, prep):
                st = get_state(gc)
                c0, cn = calls[q][st["next_call"][q]]
                mg = mgp.tile([P, GCH, P], BF16, tag="mg",
                              name=f"mg_{gc}_{q}_{c0}")
                qn = st["ncalls"] % n_queues
                kw = dict(prepare_only=True, sem=qsems[qn]) if prep else {}
                nc.gpsimd.dma_gather(
                    out_ap=mg[:, :cn, :],
                    in_ap=src_tab_for(gc)[q][int(BASE_q[q]):, :],
                    idxs_ap=idx_sb[:, c0 * 8:(c0 + cn) * 8],
                    num_idxs=cn * P,
                    num_idxs_reg=cn * P,
                    elem_size=EMB,
                    single_packet=single_packet,
                    queue_num=qn,
                    **kw,
                )
                if prep:
                    st["trig"].add(qn)
                st["ncalls"] += 1
                for j in range(cn):
                    st["chunk2mg"][c0 + j] = (mg, j)
                st["next_call"][q] += 1

            for gc in range(reps * n_convs):
                r, c = divmod(gc, n_convs)
                l = min(c // NUM_CONVS, L - 1)
                hT_cur = hT_pp[gc % 2]
                hT_nxt = hT_pp[(gc + 1) % 2]
                last = gc == reps * n_convs - 1

                st_gc = get_state(gc)
                # fire any descriptors pre-generated at the previous boundary
                for qn in sorted(st_gc["trig"]):
                    nc.gpsimd.trigger_dma(count=None, queue_num=qn)
                st_gc["trig"].clear()
                chunk2mg = st_gc["chunk2mg"]

                def issue_calls(q, need_end):
                    st = st_gc
                    while (st["next_call"][q] < len(calls[q])
                           and calls[q][st["next_call"][q]][0] < need_end):
                        issue_one_call(gc, q, False)

                def agg_block(q, b, first):
                    # psum-accumulate phase q of block b, then fold into zT
                    kb = int(K_qb[q][b])
                    bs = slice(b * P, (b + 1) * P)
                    if kb == 0:
                        if first:
                            nc.vector.tensor_copy(out=zT_all[:, bs],
                                                  in_=hT_cur[:, bs])
                        return
                    ob = int(o_qb[q][b])
                    issue_calls(q, ob + kb)
                    psumA = psA.tile([P, P], F32, space="PSUM", tag="agg",
                                     name=f"agg_{gc}_{q}_{b}")
                    n_oh = (kb + OHG - 1) // OHG
                    ohts = []
                    for j in range(n_oh):
                        k0 = j * OHG
                        kn = min(OHG, kb - k0)
                        oht = ohp.tile([P, OHG, P], BF16, tag="oh",
                                       name=f"oh_{gc}_{q}_{b}_{j}")
                        nc.vector.tensor_tensor(
                            out=oht[:, :kn, :],
                            in0=iota_sb[:, :kn * P].rearrange(
                                "p (a b) -> p a b", b=P),
                            in1=dstloc_sb[:, ob + k0:ob + k0 + kn]
                                .to_broadcast([P, kn, P]),
                            op=mybir.AluOpType.is_equal)
                        ohts.append((oht, k0, kn))
                    for oht, k0, kn in ohts:
                        for kk in range(kn):
                            k = k0 + kk
                            mg, off = chunk2mg[ob + k]
                            nc.tensor.matmul(out=psumA[:],
                                             lhsT=mg[:, off, :],
                                             rhs=oht[:, kk, :],
                                             start=(k == 0),
                                             stop=(k == kb - 1))
                    nc.vector.tensor_add(
                        out=zT_all[:, bs], in0=psumA[:],
                        in1=hT_cur[:, bs] if first else zT_all[:, bs])

                # small trailing phases first (their tables landed early via
                # the tiny tail collectives); phase 0 fused with the MLP
                for q in range(1, nchunk):
                    for b in range(NB):
                        agg_block(q, b, q == 1)

                pending_cc = []  # (chunk idx, groups countdown)
                for gi, (g0, g1) in enumerate(groups):
                    for b in range(g0, g1):
                        agg_block(0, b, nchunk == 1)
                    # grouped MLP: moving dim = 128 * (g1 - g0)
                    gw = (g1 - g0) * P
                    gsl = slice(g0 * P, g0 * P + gw)
                    z1 = []
                    for mh in range(2):
                        ps1 = psB.tile([P, 512], F32, space="PSUM", tag="mm1",
                                       name=f"mm1_{gc}_{g0}_{mh}")
                        nc.tensor.matmul(
                            out=ps1[:, :gw],
                            lhsT=w1_sb[:, (l * 2 + mh) * P:(l * 2 + mh + 1) * P],
                            rhs=zT_all[:, gsl], start=True, stop=True)
                        z1t = wp.tile([P, 512], BF16, tag=f"z1_{mh}",
                                      name=f"z1_{gc}_{g0}_{mh}")
                        nc.scalar.activation(
                            out=z1t[:, :gw], in_=ps1[:, :gw], func=Relu,
                            bias=b1_sb[:, l * 2 + mh:l * 2 + mh + 1])
                        z1.append(z1t)
                    ps2 = psM.tile([P, 512], F32, space="PSUM", tag="mm2",
                                   name=f"mm2_{gc}_{g0}")
                    for mh in range(2):
                        nc.tensor.matmul(
                            out=ps2[:, :gw],
                            lhsT=w2_sb[:, (l * 2 + mh) * P:(l * 2 + mh + 1) * P],
                            rhs=z1[mh][:, :gw], start=(mh == 0), stop=(mh == 1))
                    nc.scalar.activation(out=hT_nxt[:, gsl], in_=ps2[:, :gw],
                                         func=Relu, bias=b2_sb[:, l:l + 1])
                    # node-major h_new per block (for allgather / pooling)
                    for b in range(g0, g1):
                        bs = slice(b * P, (b + 1) * P)
                        psT = psT_pool.tile([P, P], BF16, space="PSUM", tag="tp",
                                            name=f"tp_{gc}_{b}")
                        nc.tensor.transpose(out=psT[:], in_=hT_nxt[:, bs],
                                            identity=ident_bf[:])
                        hnode = wp.tile([P, P], BF16, tag="hnode",
                                        name=f"hn_{gc}_{b}")
                        nc.scalar.copy(out=hnode[:], in_=psT[:])
                        if not last:
                            nc.sync.dma_start(hnew_pp[gc % 2][bs, :], hnode[:])
                        else:
                            nc.tensor.matmul(out=psum_pool[:],
                                             lhsT=ohg_sb[:, bs], rhs=hnode[:],
                                             start=(b == 0), stop=(b == NB - 1),
                                             skip_group_check=True)
                    # chunked allgathers, delayed a few groups so the hnew
                    # writes they wait on have drained (Pool queue-head stall)
                    if not last and not no_cc:
                        for q in range(nchunk):
                            if cb[q + 1] == g1:
                                pending_cc.append([q, cc_delay])
                        for pc in pending_cc:
                            pc[1] -= 1
                        while pending_cc and (pending_cc[0][1] < 0
                                              or gi == len(groups) - 1):
                            q = pending_cc.pop(0)[0]
                            nc.gpsimd.collective_compute(
                                "AllGather", mybir.AluOpType.bypass,
                                replica_groups=[list(range(NCORES))],
                                ins=[hnew_pp[gc % 2][int(cb[q]) * P:
                                                     int(cb[q + 1]) * P, :].opt()],
                                outs=[hfq_cv[gc][q].opt()])
                # pre-generate the next conv's first descriptors while the
                # collective runs (descriptor gen reads only idx metadata; the
                # deferred table read lands on trigger_dma)
                if not last:
                    st_nx = get_state(gc + 1)
                    for _ in range(prep_n):
                        if st_nx["next_call"][0] >= len(calls[0]):
                            break
                        issue_one_call(gc + 1, 0, True)

            # pooling epilogue
            sums_sb = cp.tile([P, P], F32)
            means_sb = cp.tile([P, P], F32)
            nc.vector.tensor_copy(out=sums_sb[:], in_=psum_pool[:])
            nc.vector.tensor_scalar(out=means_sb[:], in0=psum_pool[:],
                                    scalar1=invc_sb[:, 0:1], scalar2=None,
                                    op0=mybir.AluOpType.mult)
            psTs = psT_pool.tile([P, P], F32, space="PSUM", tag="tp")
            nc.tensor.transpose(out=psTs[:], in_=sums_sb[:], identity=ident[:])
            sT = cp.tile([P, P], F32)
            nc.scalar.copy(out=sT[:], in_=psTs[:])
            psTm = psT_pool.tile([P, P], F32, space="PSUM", tag="tp")
            nc.tensor.transpose(out=psTm[:], in_=means_sb[:], identity=ident[:])
            mT = cp.tile([P, P], F32)
            nc.scalar.copy(out=mT[:], in_=psTm[:])

            z1p = []
            for mh in range(2):
                ps3 = psB.tile([P, 512], F32, space="PSUM", tag="mm1")
                nc.tensor.matmul(out=ps3[:, :P],
                                 lhsT=pw1_sb[:, (0 * 2 + mh) * P:(0 * 2 + mh + 1) * P],
                                 rhs=sT[:], start=True, stop=False)
                nc.tensor.matmul(out=ps3[:, :P],
                                 lhsT=pw1_sb[:, (1 * 2 + mh) * P:(1 * 2 + mh + 1) * P],
                                 rhs=mT[:], start=False, stop=True)
                z1t = cp.tile([P, P], F32, name=f"z1p_{mh}")
                nc.scalar.activation(out=z1t[:], in_=ps3[:, :P], func=Relu,
                                     bias=pb1_sb[:, mh:mh + 1])
                z1p.append(z1t)
            ps4 = psM.tile([P, 512], F32, space="PSUM", tag="mm2")
            for kc in range(2):
                nc.tensor.matmul(out=ps4[:, :P], lhsT=pw2_sb[:, kc * P:(kc + 1) * P],
                                 rhs=z1p[kc][:], start=(kc == 0), stop=(kc == 1))
            out_sb = cp.tile([P, P], F32)
            nc.vector.tensor_scalar(out=out_sb[:], in0=ps4[:, :P],
                                    scalar1=pb2_sb[:, 0:1], scalar2=None,
                                    op0=mybir.AluOpType.add)
            nc.sync.dma_start(o_outT[:], out_sb[:])

    nc.compile()
    return nc


def kernel(**inputs):
    x = np.asarray(inputs["x"], np.float32)
    edge_index = np.asarray(inputs["edge_index"], np.int64)
    batch = np.asarray(inputs["batch"], np.int64)
    gin_w1 = np.asarray(inputs["gin_w1"], np.float32)
    gin_b1 = np.asarray(inputs["gin_b1"], np.float32)
    gin_w2 = np.asarray(inputs["gin_w2"], np.float32)
    gin_b2 = np.asarray(inputs["gin_b2"], np.float32)
    post_w1 = np.asarray(inputs["post_w1"], np.float32)
    post_b1 = np.asarray(inputs["post_b1"], np.float32)
    post_w2 = np.asarray(inputs["post_w2"], np.float32)
    post_b2 = np.asarray(inputs["post_b2"], np.float32)

    nchunk = int(os.environ.get("GNN_NCHUNK", "1"))
    MLPG = 4
    GCH = int(os.environ.get("GNN_GCH", "8"))
    geom, per_core, h0q = _preprocess(x, edge_index, batch, nchunk, MLPG, GCH)
    w = _pack_weights(gin_w1, gin_b1, gin_w2, gin_b2, post_w1, post_b1,
                      post_w2, post_b2)

    n_convs = int(os.environ.get("GNN_CONVS", L * NUM_CONVS))
    nc = _build_program(geom, n_convs, reps=int(os.environ.get('GNN_REPS', '1')))

    NPAD = geom["NPAD"]
    ns = geom["ns"]
    iota_np = np.tile(np.arange(128, dtype=np.float32),
                      (128, int(os.environ.get("GNN_OHG", "4"))))
    w1_bf = w["w1"].astype(ml_dtypes.bfloat16)
    w2_bf = w["w2"].astype(ml_dtypes.bfloat16)
    in_maps = []
    for c in range(NCORES):
        pc = per_core[c]
        h0T = np.zeros((P, NPAD), np.float32)
        n_c = int(ns[c + 1] - ns[c])
        h0T[:NF, :n_c] = x[ns[c]:ns[c + 1]].T
        im = {
            "t_h0T": h0T.astype(ml_dtypes.bfloat16), "t_idx": pc["idx"],
            "t_dstloc": pc["dstloc"], "t_iota": iota_np, "t_ohg": pc["ohg"],
            "t_invc": pc["invc"], "t_w1": w1_bf, "t_w2": w2_bf,
            "t_b1": w["b1"], "t_b2": w["b2"], "t_pw1": w["pw1"],
            "t_pw2": w["pw2"], "t_pb1": w["pb1"], "t_pb2": w["pb2"],
        }
        for q in range(nchunk):
            im[f"t_h0q{q}"] = h0q[q]
        in_maps.append(im)

    from concourse.bass_utils import run_bass_kernel_spmd
    trace = os.environ.get("GNN_TRACE", "0") == "1"
    res = run_bass_kernel_spmd(nc, in_maps, core_ids=list(range(NCORES)),
                               trace=trace)
    if trace:
        kernel.last_results = res
        if os.environ.get("GNN_TRACE_QUIET", "0") != "1":
            print(f"HW exec time: {res.exec_time_ns} ns")

    gs = geom["gs"]
    out = np.zeros((G, EMB), np.float32)
    for c in range(NCORES):
        outT = res.results[c]["o_outT"]  # [emb, graph slots]
        ng = per_core[c]["ng"]
        out[gs[c]:gs[c] + ng] = outT[:, :ng].T
    return out
